# revision 8
# baseline (speedup 1.0000x reference)
"""Trainium2 Bass kernel for the soft-decision-tree ensemble classifier.

Math (per batch row b, tree t):
  zb[t,n]      = x[b] . W[t,n] + bias[t,n]
  log s        = zb - softplus(zb);  log(1-s) = -softplus(zb)
  log_leaf[l]  = sum_{k in path(l)} dir_k * zb_k  -  sum_{k in path(l)} softplus(zb_k)
  leaf_prob    = exp(log_leaf)
  out[b,c]     = sum_l leaf_prob[t,l] * V[t,l,c],   V = 2*softmax(tw)_t*softmax(leaf_logits[t,l])

Mapping: data-parallel over the batch (B=4096 -> 512 rows per NeuronCore).
Per core, logits live in [tree-node (padded 64/tree), batch] layout so the
per-tree path sums become 128-wide matmuls with +/-1 constant matrices
(block-diagonal over a pair of trees per 128-partition tile).  Both
softmaxes (tree weights, leaf distributions) are folded into V on the HOST,
so the device only runs: stage-1 z matmuls, softplus (Exp+Ln on ACT, one
pinned table), path-sum matmuls, exp, and the output matmul.

The 32 node-tiles are processed in groups (2,4,4,...,4,1,1).  Per group the
ACT work is 3 big instructions (Exp over the group's z, Ln, Exp over the
group's path sums read straight from PSUM).  Stage-2 matmuls are deferred
one group and stage-3 (output) matmuls two groups, so the PE queue never
head-of-line blocks on the ACT chain.  Input DMAs are chunked in
consumption order and spread across the SP and POOL queues (not ACT/DVE,
which are busy).
"""

import numpy as np

TREE_DEPTH = 6
T, N, D, C = 64, 63, 512, 100
L = 2**TREE_DEPTH          # 64
NPAD = 64                  # nodes padded per tree
TNP = T * NPAD             # 4096
NTILES = TNP // 128        # 32 (two trees per 128-partition tile)
B = 4096
NCORES = 8
BS = B // NCORES           # 512

GROUPS = [1, 2, 4, 4, 4, 4, 4, 4, 3, 1, 1]
assert sum(GROUPS) == NTILES


def _leaf_paths(depth):
    Ll = 2**depth
    idx = np.zeros((Ll, depth), np.int32)
    dr = np.zeros((Ll, depth), np.int32)
    for l in range(Ll):
        node = 0
        for k in range(depth):
            bit = (l >> (depth - 1 - k)) & 1
            idx[l, k] = node
            dr[l, k] = bit
            node = 2 * node + 1 + bit
    return idx, dr


def _path_mats():
    """[128, 256] fp16: block-diag (2 trees) dir matrix | path matrix."""
    idx, dr = _leaf_paths(TREE_DEPTH)
    mdir = np.zeros((NPAD, L), np.float32)   # [node, leaf] +1 where dir=1
    mpath = np.zeros((NPAD, L), np.float32)  # [node, leaf] -1 on path
    for l in range(L):
        for k in range(TREE_DEPTH):
            n = idx[l, k]
            mpath[n, l] -= 1.0
            if dr[l, k]:
                mdir[n, l] += 1.0
    amat = np.zeros((128, 256), np.float16)
    amat[:NPAD, 0:L] = mdir
    amat[NPAD:, L:128] = mdir
    amat[:NPAD, 128:128 + L] = mpath
    amat[NPAD:, 128 + L:256] = mpath
    return amat


_NC_CACHE = {}


def _build_bass():
    import concourse.bacc as bacc
    import concourse.mybir as mybir
    import concourse.tile as tile
    from concourse.hw_specs import get_activation_tables

    dt = mybir.dt
    f32 = dt.float32
    fp16 = dt.float16
    AF = mybir.ActivationFunctionType

    nc = bacc.Bacc("TRN2", target_bir_lowering=False, debug=False,
                   num_devices=NCORES)

    # Pin the ACT function table to the one containing BOTH Exp and Ln, else
    # the table-load pass ping-pongs between single-function tables (~2.7us
    # per reload).
    table_id = next(i for i, (_, funcs) in
                    enumerate(get_activation_tables("gen3").items())
                    if AF.Exp in funcs and AF.Ln in funcs)
    nc.scalar.add_instruction(mybir.InstLoadActFuncSet(
        name=f"I-{nc.next_id()}", ins=[], outs=[], act_func_set_id=table_id))

    xt = nc.dram_tensor("xt", [D, BS], fp16, kind="ExternalInput").ap()
    wt = nc.dram_tensor("wt", [D, TNP], fp16, kind="ExternalInput").ap()
    bias = nc.dram_tensor("bias", [128, NTILES], f32,
                          kind="ExternalInput").ap()
    amat = nc.dram_tensor("amat", [128, 256], fp16, kind="ExternalInput").ap()
    vt = nc.dram_tensor("vt", [TNP, C], fp16, kind="ExternalInput").ap()
    out = nc.dram_tensor("out", [C, BS], f32, kind="ExternalOutput").ap()

    with tile.TileContext(nc) as tc:
        with (
            tc.tile_pool(name="big", bufs=1) as bigp,
            tc.tile_pool(name="const", bufs=1) as constp,
            tc.tile_pool(name="work", bufs=3) as work,
            tc.tile_pool(name="lpp", bufs=3) as lpp,
            tc.tile_pool(name="pz", bufs=3, space="PSUM") as pzp,
            tc.tile_pool(name="pp", bufs=1, space="PSUM") as ppp,
            tc.tile_pool(name="po", bufs=1, space="PSUM") as pop,
        ):
            # ---- input tiles ------------------------------------------
            wt_t = [bigp.tile([128, TNP], fp16, tag=f"wt{j}", name=f"wt{j}")
                    for j in range(4)]
            xt_t = bigp.tile([128, 4 * BS], fp16, tag="xt")
            vt_t = bigp.tile([128, NTILES * C], fp16, tag="vt")
            bias_t = constp.tile([128, NTILES], f32, tag="bias")
            amat_t = constp.tile([128, 256], fp16, tag="amat")

            # DMAs in consumption order, spread over SP + POOL queues.
            # col chunks of wt: tiles 0-3 / 4-11 / 12-19 / 20-31
            CC = [(0, 512), (512, 1536), (1536, 2560), (2560, 4096)]

            def dma_xt(eng, j):
                eng.dma_start(out=xt_t[:, j * BS:(j + 1) * BS],
                              in_=xt[j * 128:(j + 1) * 128, :])

            def dma_wt(eng, j, ci):
                c0, c1 = CC[ci]
                eng.dma_start(out=wt_t[j][:, c0:c1],
                              in_=wt[j * 128:(j + 1) * 128, c0:c1])

            # round 1-2: everything the first tiles need, one kick per queue
            # (vector/scalar queues are idle until ~10us; their early kicks
            # execute immediately and precede their first compute ops)
            dma_xt(nc.sync, 0)
            dma_wt(nc.gpsimd, 0, 0)
            dma_wt(nc.scalar, 1, 0)
            dma_xt(nc.sync, 1)
            dma_xt(nc.gpsimd, 2)
            dma_xt(nc.scalar, 3)
            dma_wt(nc.sync, 2, 0)
            dma_wt(nc.gpsimd, 3, 0)
            nc.scalar.dma_start(out=bias_t[:], in_=bias[:])
            nc.sync.dma_start(out=amat_t[:], in_=amat[:])
            nc.gpsimd.dma_start(
                out=vt_t[:].rearrange("p (i c) -> p i c", c=C),
                in_=vt.rearrange("(i p) c -> p i c", p=128),
            )
            dma_wt(nc.sync, 0, 1)
            dma_wt(nc.gpsimd, 1, 1)
            dma_wt(nc.scalar, 3, 3)
            dma_wt(nc.sync, 2, 1)
            dma_wt(nc.gpsimd, 3, 1)
            dma_wt(nc.sync, 0, 2)
            dma_wt(nc.gpsimd, 1, 2)
            dma_wt(nc.sync, 2, 2)
            dma_wt(nc.gpsimd, 3, 2)
            dma_wt(nc.sync, 0, 3)
            dma_wt(nc.gpsimd, 1, 3)
            dma_wt(nc.sync, 2, 3)

            adir_ap = amat_t[:, 0:128]
            apath_ap = amat_t[:, 128:256]

            # ---- main pipeline ----------------------------------------
            out_ps = pop.tile([C, BS], f32, tag="outps")
            s2_prev = None         # stage-2 closure of group g-1 (not run)
            s3_prev = None         # stage-3 closure of group g-2 (not run)

            base = 0
            for gi, gsz in enumerate(GROUPS):
                gbase = base
                ta = work.tile([128, 2048], fp16, tag="ta", name=f"ta{gi}")
                te = work.tile([128, 2048], fp16, tag="te", name=f"te{gi}")
                tb = work.tile([128, 2048], fp16, tag="tb", name=f"tb{gi}")
                # stage 1: z matmuls + PSUM->SBUF (bias-add) drain
                for k in range(gsz):
                    i = gbase + k
                    pz = pzp.tile([128, BS], f32, tag="pz", name=f"pz{i}")
                    for j in range(4):
                        nc.tensor.matmul(
                            pz[:],
                            lhsT=wt_t[j][:, i * 128:(i + 1) * 128],
                            rhs=xt_t[:, j * BS:(j + 1) * BS],
                            start=(j == 0), stop=(j == 3),
                        )
                    nc.vector.tensor_scalar_add(
                        out=ta[:, k * BS:(k + 1) * BS], in0=pz[:],
                        scalar1=bias_t[:, i:i + 1])

                # stage-2 of group g-1 (incl. its Exp(pp)), then stage-3 of
                # group g-2 — two-level deferral so the PE queue never
                # head-of-line blocks on this group's ACT chain.
                s3_new = s2_prev() if s2_prev is not None else None
                if s3_prev is not None:
                    s3_prev()

                # softplus chain for this group (2 big ACT instructions)
                nc.scalar.activation(te[:, 0:gsz * BS], ta[:, 0:gsz * BS],
                                     AF.Exp)
                nc.scalar.activation(tb[:, 0:gsz * BS], te[:, 0:gsz * BS],
                                     AF.Ln, bias=1.0, scale=1.0)

                def stage2(gbase=gbase, gsz=gsz, ta=ta, tb=tb, gi=gi):
                    pp = ppp.tile([128, 2048], f32, tag="pp", name=f"pp{gi}")
                    lp = lpp.tile([128, 2048], fp16, tag="lp", name=f"lp{gi}")
                    for k in range(gsz):
                        sl = slice(k * BS, (k + 1) * BS)
                        nc.tensor.matmul(pp[:, sl], lhsT=adir_ap,
                                         rhs=ta[:, sl],
                                         start=True, stop=False)
                    for k in range(gsz):
                        sl = slice(k * BS, (k + 1) * BS)
                        nc.tensor.matmul(pp[:, sl], lhsT=apath_ap,
                                         rhs=tb[:, sl],
                                         start=False, stop=True)
                    nc.scalar.activation(lp[:, 0:gsz * BS],
                                         pp[:, 0:gsz * BS], AF.Exp)

                    def stage3():
                        for k in range(gsz):
                            ii = gbase + k
                            nc.tensor.matmul(
                                out_ps[:],
                                lhsT=vt_t[:, ii * C:(ii + 1) * C],
                                rhs=lp[:, k * BS:(k + 1) * BS],
                                start=(ii == 0),
                                stop=(ii == NTILES - 1))
                    return stage3

                s2_prev = stage2
                s3_prev = s3_new
                base += gsz

            # drain the pipeline: stage3(g-2), stage2(g-1)+stage3(g-1)
            if s3_prev is not None:
                s3_prev()
            s2_prev()()

            out_sb = work.tile([C, BS], f32, tag="osb")
            nc.vector.tensor_copy(out=out_sb[:], in_=out_ps[:])
            nc.sync.dma_start(out=out[:], in_=out_sb[:])

    nc.finalize()
    return nc


def _get_nc():
    if "nc" not in _NC_CACHE:
        _NC_CACHE["nc"] = _build_bass()
    return _NC_CACHE["nc"]


def _prep_inputs(x, split_weights, split_bias, leaf_logits, tree_weights):
    x = np.asarray(x, np.float32)
    split_weights = np.asarray(split_weights, np.float32)
    split_bias = np.asarray(split_bias, np.float32)
    leaf_logits = np.asarray(leaf_logits, np.float64)
    tree_weights = np.asarray(tree_weights, np.float64)

    wpad = np.zeros((T, NPAD, D), np.float32)
    wpad[:, :N, :] = split_weights
    wtT = np.ascontiguousarray(
        wpad.reshape(TNP, D).T.astype(np.float16))              # [D, TNP]

    bpad = np.zeros((T, NPAD), np.float32)
    bpad[:, :N] = split_bias
    bias = np.ascontiguousarray(
        bpad.reshape(NTILES, 128).T.astype(np.float32))         # [128, 32]

    # host-side: fold both softmaxes + the factor 2 into the leaf dists
    tw = np.exp(tree_weights - tree_weights.max())
    tw = tw / tw.sum()                                          # [T]
    ll = leaf_logits - leaf_logits.max(axis=-1, keepdims=True)
    ev = np.exp(ll)
    sm = ev / ev.sum(axis=-1, keepdims=True)                    # [T, L, C]
    vt = (2.0 * tw[:, None, None] * sm).reshape(TNP, C)
    vt = np.ascontiguousarray(vt.astype(np.float16))            # [TNP, C]

    amat = _path_mats()
    shared = dict(wt=wtT, bias=bias, amat=amat, vt=vt)
    in_maps = []
    for i in range(NCORES):
        xti = np.ascontiguousarray(
            x[i * BS:(i + 1) * BS, :].T.astype(np.float16))     # [D, BS]
        in_maps.append(dict(xt=xti, **shared))
    return in_maps


def kernel(x, split_weights, split_bias, leaf_logits, tree_weights):
    from concourse.bass_utils import run_bass_kernel_spmd

    in_maps = _prep_inputs(x, split_weights, split_bias, leaf_logits,
                           tree_weights)
    nc = _get_nc()
    res = run_bass_kernel_spmd(nc, in_maps, core_ids=list(range(NCORES)))
    out = np.concatenate([res.results[i]["out"] for i in range(NCORES)],
                         axis=1).T                              # [B, C]
    return np.ascontiguousarray(out.astype(np.float32))


# revision 9
# speedup vs baseline: 1.0153x; 1.0153x over previous
"""Trainium2 Bass kernel for the soft-decision-tree ensemble classifier.

Math (per batch row b, tree t):
  zb[t,n]      = x[b] . W[t,n] + bias[t,n]
  log s        = zb - softplus(zb);  log(1-s) = -softplus(zb)
  log_leaf[l]  = sum_{k in path(l)} dir_k * zb_k  -  sum_{k in path(l)} softplus(zb_k)
  leaf_prob    = exp(log_leaf)
  out[b,c]     = sum_l leaf_prob[t,l] * V[t,l,c],   V = 2*softmax(tw)_t*softmax(leaf_logits[t,l])

Mapping: data-parallel over the batch (B=4096 -> 512 rows per NeuronCore).
Per core, logits live in [tree-node (padded 64/tree), batch] layout so the
per-tree path sums become 128-wide matmuls with +/-1 constant matrices
(block-diagonal over a pair of trees per 128-partition tile).  Both
softmaxes (tree weights, leaf distributions) are folded into V on the HOST,
so the device only runs: stage-1 z matmuls, softplus (Exp+Ln on ACT, one
pinned table), path-sum matmuls, exp, and the output matmul.

The 32 node-tiles are processed in groups (2,4,4,...,4,1,1).  Per group the
ACT work is 3 big instructions (Exp over the group's z, Ln, Exp over the
group's path sums read straight from PSUM).  Stage-2 matmuls are deferred
one group and stage-3 (output) matmuls two groups, so the PE queue never
head-of-line blocks on the ACT chain.  Input DMAs are chunked in
consumption order and spread across the SP and POOL queues (not ACT/DVE,
which are busy).
"""

import numpy as np

TREE_DEPTH = 6
T, N, D, C = 64, 63, 512, 100
L = 2**TREE_DEPTH          # 64
NPAD = 64                  # nodes padded per tree
TNP = T * NPAD             # 4096
NTILES = TNP // 128        # 32 (two trees per 128-partition tile)
B = 4096
NCORES = 8
BS = B // NCORES           # 512

GROUPS = [1, 2, 4, 4, 4, 4, 4, 4, 3, 1, 1]
assert sum(GROUPS) == NTILES


def _leaf_paths(depth):
    Ll = 2**depth
    idx = np.zeros((Ll, depth), np.int32)
    dr = np.zeros((Ll, depth), np.int32)
    for l in range(Ll):
        node = 0
        for k in range(depth):
            bit = (l >> (depth - 1 - k)) & 1
            idx[l, k] = node
            dr[l, k] = bit
            node = 2 * node + 1 + bit
    return idx, dr


def _path_mats():
    """[128, 256] fp16: block-diag (2 trees) dir matrix | path matrix."""
    idx, dr = _leaf_paths(TREE_DEPTH)
    mdir = np.zeros((NPAD, L), np.float32)   # [node, leaf] +1 where dir=1
    mpath = np.zeros((NPAD, L), np.float32)  # [node, leaf] -1 on path
    for l in range(L):
        for k in range(TREE_DEPTH):
            n = idx[l, k]
            mpath[n, l] -= 1.0
            if dr[l, k]:
                mdir[n, l] += 1.0
    amat = np.zeros((128, 256), np.float16)
    amat[:NPAD, 0:L] = mdir
    amat[NPAD:, L:128] = mdir
    amat[:NPAD, 128:128 + L] = mpath
    amat[NPAD:, 128 + L:256] = mpath
    return amat


_NC_CACHE = {}


def _build_bass():
    import concourse.bacc as bacc
    import concourse.mybir as mybir
    import concourse.tile as tile
    from concourse.hw_specs import get_activation_tables

    dt = mybir.dt
    f32 = dt.float32
    fp16 = dt.float16
    AF = mybir.ActivationFunctionType

    nc = bacc.Bacc("TRN2", target_bir_lowering=False, debug=False,
                   num_devices=NCORES)

    # Pin the ACT function table to the one containing BOTH Exp and Ln, else
    # the table-load pass ping-pongs between single-function tables (~2.7us
    # per reload).
    table_id = next(i for i, (_, funcs) in
                    enumerate(get_activation_tables("gen3").items())
                    if AF.Exp in funcs and AF.Ln in funcs)
    nc.scalar.add_instruction(mybir.InstLoadActFuncSet(
        name=f"I-{nc.next_id()}", ins=[], outs=[], act_func_set_id=table_id))

    xt = nc.dram_tensor("xt", [D, BS], fp16, kind="ExternalInput").ap()
    wt = nc.dram_tensor("wt", [D, TNP], fp16, kind="ExternalInput").ap()
    bias = nc.dram_tensor("bias", [128, NTILES], f32,
                          kind="ExternalInput").ap()
    amat = nc.dram_tensor("amat", [128, 256], fp16, kind="ExternalInput").ap()
    vt = nc.dram_tensor("vt", [TNP, C], fp16, kind="ExternalInput").ap()
    out = nc.dram_tensor("out", [C, BS], f32, kind="ExternalOutput").ap()

    with tile.TileContext(nc) as tc:
        with (
            tc.tile_pool(name="big", bufs=1) as bigp,
            tc.tile_pool(name="const", bufs=1) as constp,
            tc.tile_pool(name="work", bufs=3) as work,
            tc.tile_pool(name="lpp", bufs=3) as lpp,
            tc.tile_pool(name="pz", bufs=3, space="PSUM") as pzp,
            tc.tile_pool(name="pp", bufs=1, space="PSUM") as ppp,
            tc.tile_pool(name="po", bufs=1, space="PSUM") as pop,
        ):
            # ---- input tiles ------------------------------------------
            wt_t = [bigp.tile([128, TNP], fp16, tag=f"wt{j}", name=f"wt{j}")
                    for j in range(4)]
            xt_t = bigp.tile([128, 4 * BS], fp16, tag="xt")
            vt_t = bigp.tile([128, NTILES * C], fp16, tag="vt")
            bias_t = constp.tile([128, NTILES], f32, tag="bias")
            amat_t = constp.tile([128, 256], fp16, tag="amat")

            # DMAs in consumption order, spread over SP + POOL queues.
            # col chunks of wt: tiles 0-3 / 4-11 / 12-19 / 20-31
            CC = [(0, 512), (512, 1536), (1536, 2560), (2560, 4096)]

            def dma_xt(eng, j):
                eng.dma_start(out=xt_t[:, j * BS:(j + 1) * BS],
                              in_=xt[j * 128:(j + 1) * 128, :])

            def dma_wt(eng, j, ci):
                c0, c1 = CC[ci]
                eng.dma_start(out=wt_t[j][:, c0:c1],
                              in_=wt[j * 128:(j + 1) * 128, c0:c1])

            # round 1-2: everything the first tiles need, one kick per queue
            # (vector/scalar queues are idle until ~10us; their early kicks
            # execute immediately and precede their first compute ops)
            vt_pic = vt_t[:].rearrange("p (i c) -> p i c", c=C)
            vt_src = vt.rearrange("(i p) c -> p i c", p=128)
            dma_xt(nc.sync, 0)
            dma_wt(nc.gpsimd, 0, 0)
            dma_wt(nc.scalar, 3, 0)
            dma_wt(nc.sync, 1, 0)
            dma_xt(nc.gpsimd, 1)
            dma_xt(nc.scalar, 3)
            dma_xt(nc.sync, 2)
            dma_wt(nc.gpsimd, 2, 0)
            nc.scalar.dma_start(out=bias_t[:], in_=bias[:])
            nc.sync.dma_start(out=amat_t[:], in_=amat[:])
            dma_wt(nc.gpsimd, 1, 1)
            nc.scalar.dma_start(out=vt_pic[:, 8:NTILES, :],
                                in_=vt_src[:, 8:NTILES, :])
            dma_wt(nc.sync, 0, 1)
            dma_wt(nc.gpsimd, 3, 1)
            dma_wt(nc.sync, 2, 1)
            nc.gpsimd.dma_start(out=vt_pic[:, 0:8, :], in_=vt_src[:, 0:8, :])
            dma_wt(nc.sync, 0, 2)
            dma_wt(nc.gpsimd, 1, 2)
            dma_wt(nc.sync, 2, 2)
            dma_wt(nc.gpsimd, 3, 2)
            dma_wt(nc.sync, 0, 3)
            dma_wt(nc.gpsimd, 1, 3)
            dma_wt(nc.sync, 2, 3)
            dma_wt(nc.gpsimd, 3, 3)

            adir_ap = amat_t[:, 0:128]
            apath_ap = amat_t[:, 128:256]

            # ---- main pipeline ----------------------------------------
            out_ps = pop.tile([C, BS], f32, tag="outps")
            s2_prev = None         # stage-2 closure of group g-1 (not run)
            s3_prev = None         # stage-3 closure of group g-2 (not run)

            base = 0
            for gi, gsz in enumerate(GROUPS):
                gbase = base
                ta = work.tile([128, 2048], fp16, tag="ta", name=f"ta{gi}")
                te = work.tile([128, 2048], fp16, tag="te", name=f"te{gi}")
                tb = work.tile([128, 2048], fp16, tag="tb", name=f"tb{gi}")
                # stage 1: z matmuls + PSUM->SBUF (bias-add) drain
                for k in range(gsz):
                    i = gbase + k
                    pz = pzp.tile([128, BS], f32, tag="pz", name=f"pz{i}")
                    for j in range(4):
                        nc.tensor.matmul(
                            pz[:],
                            lhsT=wt_t[j][:, i * 128:(i + 1) * 128],
                            rhs=xt_t[:, j * BS:(j + 1) * BS],
                            start=(j == 0), stop=(j == 3),
                        )
                    nc.vector.tensor_scalar_add(
                        out=ta[:, k * BS:(k + 1) * BS], in0=pz[:],
                        scalar1=bias_t[:, i:i + 1])

                # stage-2 of group g-1 (incl. its Exp(pp)), then stage-3 of
                # group g-2 — two-level deferral so the PE queue never
                # head-of-line blocks on this group's ACT chain.
                s3_new = s2_prev() if s2_prev is not None else None
                if s3_prev is not None:
                    s3_prev()

                # softplus chain for this group (2 big ACT instructions)
                nc.scalar.activation(te[:, 0:gsz * BS], ta[:, 0:gsz * BS],
                                     AF.Exp)
                nc.scalar.activation(tb[:, 0:gsz * BS], te[:, 0:gsz * BS],
                                     AF.Ln, bias=1.0, scale=1.0)

                def stage2(gbase=gbase, gsz=gsz, ta=ta, tb=tb, gi=gi):
                    pp = ppp.tile([128, 2048], f32, tag="pp", name=f"pp{gi}")
                    lp = lpp.tile([128, 2048], fp16, tag="lp", name=f"lp{gi}")
                    for k in range(gsz):
                        sl = slice(k * BS, (k + 1) * BS)
                        nc.tensor.matmul(pp[:, sl], lhsT=adir_ap,
                                         rhs=ta[:, sl],
                                         start=True, stop=False)
                    for k in range(gsz):
                        sl = slice(k * BS, (k + 1) * BS)
                        nc.tensor.matmul(pp[:, sl], lhsT=apath_ap,
                                         rhs=tb[:, sl],
                                         start=False, stop=True)
                    nc.scalar.activation(lp[:, 0:gsz * BS],
                                         pp[:, 0:gsz * BS], AF.Exp)

                    def stage3():
                        for k in range(gsz):
                            ii = gbase + k
                            nc.tensor.matmul(
                                out_ps[:],
                                lhsT=vt_t[:, ii * C:(ii + 1) * C],
                                rhs=lp[:, k * BS:(k + 1) * BS],
                                start=(ii == 0),
                                stop=(ii == NTILES - 1))
                    return stage3

                s2_prev = stage2
                s3_prev = s3_new
                base += gsz

            # drain the pipeline: stage3(g-2), stage2(g-1)+stage3(g-1)
            if s3_prev is not None:
                s3_prev()
            s2_prev()()

            out_sb = work.tile([C, BS], f32, tag="osb")
            nc.vector.tensor_copy(out=out_sb[:], in_=out_ps[:])
            nc.sync.dma_start(out=out[:], in_=out_sb[:])

    nc.finalize()
    return nc


def _get_nc():
    if "nc" not in _NC_CACHE:
        _NC_CACHE["nc"] = _build_bass()
    return _NC_CACHE["nc"]


def _prep_inputs(x, split_weights, split_bias, leaf_logits, tree_weights):
    x = np.asarray(x, np.float32)
    split_weights = np.asarray(split_weights, np.float32)
    split_bias = np.asarray(split_bias, np.float32)
    leaf_logits = np.asarray(leaf_logits, np.float64)
    tree_weights = np.asarray(tree_weights, np.float64)

    wpad = np.zeros((T, NPAD, D), np.float32)
    wpad[:, :N, :] = split_weights
    wtT = np.ascontiguousarray(
        wpad.reshape(TNP, D).T.astype(np.float16))              # [D, TNP]

    bpad = np.zeros((T, NPAD), np.float32)
    bpad[:, :N] = split_bias
    bias = np.ascontiguousarray(
        bpad.reshape(NTILES, 128).T.astype(np.float32))         # [128, 32]

    # host-side: fold both softmaxes + the factor 2 into the leaf dists
    tw = np.exp(tree_weights - tree_weights.max())
    tw = tw / tw.sum()                                          # [T]
    ll = leaf_logits - leaf_logits.max(axis=-1, keepdims=True)
    ev = np.exp(ll)
    sm = ev / ev.sum(axis=-1, keepdims=True)                    # [T, L, C]
    vt = (2.0 * tw[:, None, None] * sm).reshape(TNP, C)
    vt = np.ascontiguousarray(vt.astype(np.float16))            # [TNP, C]

    amat = _path_mats()
    shared = dict(wt=wtT, bias=bias, amat=amat, vt=vt)
    in_maps = []
    for i in range(NCORES):
        xti = np.ascontiguousarray(
            x[i * BS:(i + 1) * BS, :].T.astype(np.float16))     # [D, BS]
        in_maps.append(dict(xt=xti, **shared))
    return in_maps


def kernel(x, split_weights, split_bias, leaf_logits, tree_weights):
    from concourse.bass_utils import run_bass_kernel_spmd

    in_maps = _prep_inputs(x, split_weights, split_bias, leaf_logits,
                           tree_weights)
    nc = _get_nc()
    res = run_bass_kernel_spmd(nc, in_maps, core_ids=list(range(NCORES)))
    out = np.concatenate([res.results[i]["out"] for i in range(NCORES)],
                         axis=1).T                              # [B, C]
    return np.ascontiguousarray(out.astype(np.float32))
